# revision 25
# baseline (speedup 1.0000x reference)
"""ContextualAttention2D Trainium2 kernel.

Full inputs -> full output; internally data-parallel over batch across 8
NeuronCores (2 batches per core), single SPMD NEFF, no collectives.

Math (per batch):
  hidden[n,c]   = x.reshape(C, H*W).T
  hn            = layernorm_c(hidden) * ln_w + ln_b
  q             = hn @ (Wq/8).T ;  k = ctx @ Wk.T ; v = ctx @ Wv.T
  ctx           = context @ Wctx.T      (folded: k = context @ (Wk@Wctx).T etc)
  attn          = softmax_l(q @ k.T + maskbias) ; out = attn @ v
  y             = (out @ Wo.T + hidden).T.reshape(C, H, W)

On-chip layouts are feature-on-partition ("T" = transposed, [feat, tok]):
  x_sb   [128, 4cc, 1024m] fp32 (+ bf16 copy)     QT  [128, (ec,mc)] tiles
  ctxT   [128, 6dc, 512l]  bf16                   KT  [128, ec] tiles [128,512l]
  V      l-major [128l, 8h, 65] bf16 (col 64 = ones -> softmax denominator)
  scores sT [128l, 512m] PSUM per (h,lc,mc); exp on ACT with per-partition
  mask bias; attn@V accumulates [65, 512m]; row 64 = denominator. Normalize
  via DVE mult with DMA-broadcast reciprocal; out-proj back to C-major and
  fp32 residual add with raw x.

LayerNorm: per-token mean/var via ones-matmuls (cross-partition sum), the
affine+scale folded on host into WqT', mean/rstd applied as a rank-2
PSUM-accumulated correction matmul plus an rstd scale fused into Q eviction.
"""
import numpy as np
import ml_dtypes

from concourse import bacc, mybir, tile
from concourse.bass_utils import run_bass_kernel_spmd

BF = ml_dtypes.bfloat16

B, C, H, W = 16, 512, 32, 32
NH, HD = 8, 64
CTX_DIM, L = 768, 512
EPS = 1e-5
N = H * W                 # 1024 tokens
NCORES = 8
BPC = B // NCORES         # batches per core
P = 128
CC = C // P               # 4 c-chunks
DC = CTX_DIM // P         # 6 d-chunks
LC = L // P               # 4 l-chunks
MC = N // 512             # 2 token chunks of 512
MASK_NEG = -30000.0

F32 = mybir.dt.float32
BF16 = mybir.dt.bfloat16

_NC_CACHE = None


def _build():
    nc = bacc.Bacc(None, target_bir_lowering=False, debug=False)

    xd = nc.dram_tensor("x", [BPC, C, N], F32, kind="ExternalInput")
    xbfd = nc.dram_tensor("xbf", [BPC, C, N], BF16, kind="ExternalInput")
    ctxtd = nc.dram_tensor("ctxt", [BPC, CTX_DIM, L], BF16, kind="ExternalInput")
    mbd = nc.dram_tensor("mb", [BPC, L], F32, kind="ExternalInput")
    wqd = nc.dram_tensor("wq_t", [C, C], BF16, kind="ExternalInput")
    wckd = nc.dram_tensor("wck_t", [CTX_DIM, C], BF16, kind="ExternalInput")
    wcvd = nc.dram_tensor("wcv_t", [CTX_DIM, C], BF16, kind="ExternalInput")
    wod = nc.dram_tensor("wo_t", [C, C], BF16, kind="ExternalInput")
    qr2d = nc.dram_tensor("q_r2", [2, C], BF16, kind="ExternalInput")
    yd = nc.dram_tensor("y", [BPC, C, N], F32, kind="ExternalOutput")

    with tile.TileContext(nc) as tc:
        with (
            tc.tile_pool(name="wpool", bufs=1) as wpool,
            tc.tile_pool(name="xpool", bufs=2) as xpool,
            tc.tile_pool(name="actpool", bufs=2) as actpool,
            tc.tile_pool(name="ppool", bufs=8) as ppool,
            tc.tile_pool(name="spool", bufs=2) as spool,
            tc.tile_pool(name="psum", bufs=2, space="PSUM") as psum,
            tc.tile_pool(name="psc", bufs=2, space="PSUM") as psc,
            tc.tile_pool(name="paug", bufs=2, space="PSUM") as paug,
            tc.tile_pool(name="dpool", bufs=4, space="DRAM") as dpool,
        ):
            # ---- persistent weights ----
            wq_sb = wpool.tile([P, CC, C], BF16)
            nc.scalar.dma_start(wq_sb[:], wqd.ap().rearrange("(cc p) e -> p cc e", p=P))
            wck_sb = wpool.tile([P, DC, C], BF16)
            nc.scalar.dma_start(wck_sb[:], wckd.ap().rearrange("(dc p) e -> p dc e", p=P))
            wcv_sb = wpool.tile([P, DC, C], BF16)
            nc.scalar.dma_start(wcv_sb[:], wcvd.ap().rearrange("(dc p) e -> p dc e", p=P))
            wo_sb = wpool.tile([P, CC, C], BF16)
            nc.scalar.dma_start(wo_sb[:], wod.ap().rearrange("(ec p) c -> p ec c", p=P))
            qr2_sb = wpool.tile([2, C], BF16)
            nc.scalar.dma_start(qr2_sb[:], qr2d.ap())

            ones1_sb = wpool.tile([P, 1], BF16)   # stats lhsT (column sums)
            nc.vector.memset(ones1_sb[:], 1.0)
            onesr_sb = wpool.tile([1, P], BF16)    # bcast-matmul lhsT (rank-1)
            nc.vector.memset(onesr_sb[:], 1.0)
            eps_sb = wpool.tile([1, 1], F32)
            nc.vector.memset(eps_sb[:], EPS)

            # Per-batch emission closures; emitted in a software-pipelined
            # order so PE filler (projection chains) sits between the
            # ACT-bound score-exp groups and their attn@v consumers.
            def make_batch(b):
                st = {}

                def loads():
                    # b0 bulk loads ride the sync queue; later batches use the
                    # gpsimd (SWDGE) queue so they don't delay the previous
                    # batch's latency-sensitive normalize DMAs on sync.
                    bulk = nc.sync.dma_start if b == 0 else nc.gpsimd.dma_start
                    st["x"] = xpool.tile([P, CC, N], F32, name=f"x{b}", tag="x")
                    st["xbf"] = xpool.tile([P, CC, N], BF16, name=f"xbf{b}", tag="xbf")
                    for cc in range(CC):
                        bulk(st["xbf"][:, cc, :],
                             xbfd.ap()[b][cc * P:(cc + 1) * P, :])
                    for cc in range(CC):
                        nc.scalar.dma_start(
                            st["x"][:, cc, :],
                            xd.ap()[b][cc * P:(cc + 1) * P, :])
                    st["ctxt"] = xpool.tile([P, DC, L], BF16, name=f"ctxt{b}", tag="ctxt")
                    for dc in range(DC):
                        bulk(st["ctxt"][:, dc, :],
                             ctxtd.ap()[b][dc * P:(dc + 1) * P, :])
                    st["mb"] = spool.tile([P, LC], F32, name=f"mb{b}", tag="mb")
                    nc.sync.dma_start(
                        st["mb"][:], mbd.ap()[b].rearrange("(lc p) -> p lc", p=P))
                    st["xsq"] = xpool.tile([P, CC, N], BF16, name=f"xsq{b}",
                                           tag="xsq", bufs=1)
                    for cc in range(CC):
                        nc.vector.tensor_tensor(
                            st["xsq"][:, cc, :], st["xbf"][:, cc, :],
                            st["xbf"][:, cc, :], op=mybir.AluOpType.mult)
                    st["q"] = actpool.tile([P, CC, MC, 512], BF16, name=f"q{b}", tag="q")
                    st["k"] = actpool.tile([P, CC, L], BF16, name=f"k{b}", tag="k")
                    st["v"] = actpool.tile([P, LC, NH, HD + 1], BF16,
                                           name=f"v{b}", tag="v")
                    nc.vector.memset(st["v"][:, :, :, HD:HD + 1], 1.0)
                    st["an"] = actpool.tile([P, CC, MC, 512], BF16,
                                            name=f"an{b}", tag="an")
                    st["r2"] = {}
                    st["rbc"] = {}
                    st["den8"] = {}
                    st["asb"] = {}
                    st["ps"] = {}

                def stats(mc):
                    ms = slice(mc * 512, (mc + 1) * 512)
                    st1 = psum.tile([1, 512], F32, name=f"st1{b}{mc}", tag="ps")
                    for cc in range(CC):
                        nc.tensor.matmul(st1[:], ones1_sb[:], st["xbf"][:, cc, ms],
                                         start=(cc == 0), stop=(cc == CC - 1))
                    st2 = psum.tile([1, 512], F32, name=f"st2{b}{mc}", tag="ps")
                    for cc in range(CC):
                        nc.tensor.matmul(st2[:], ones1_sb[:], st["xsq"][:, cc, ms],
                                         start=(cc == 0), stop=(cc == CC - 1))
                    negmu = spool.tile([1, 512], BF16, name=f"negmu{b}{mc}", tag="negmu")
                    nc.vector.tensor_scalar_mul(negmu[:], st1[:], -1.0 / C)
                    musq = spool.tile([1, 512], F32, name=f"musq{b}{mc}", tag="musq")
                    nc.vector.tensor_tensor(musq[:], negmu[:], negmu[:],
                                            op=mybir.AluOpType.mult)
                    var = spool.tile([1, 512], F32, name=f"var{b}{mc}", tag="var")
                    nc.vector.scalar_tensor_tensor(
                        var[:], st2[:], 1.0 / C, musq[:],
                        op0=mybir.AluOpType.mult, op1=mybir.AluOpType.subtract)
                    invr = spool.tile([1, 512], BF16, name=f"invr{b}{mc}", tag="invr")
                    nc.scalar.activation(invr[:], var[:],
                                         mybir.ActivationFunctionType.Sqrt,
                                         bias=eps_sb[:])
                    rstd = spool.tile([1, 512], BF16, name=f"rstd{b}{mc}", tag="rstd")
                    with nc.allow_low_precision(reason="softmax/LN scale rows; error damped by residual"):
                        nc.vector.reciprocal(rstd[:], invr[:])
                    r2 = spool.tile([2, 512], BF16, name=f"r2_{b}{mc}", tag="r2")
                    nc.sync.dma_start(r2[0:1, :], negmu[:])
                    nc.sync.dma_start(r2[1:2, :], invr[:])
                    rbp = paug.tile([P, 512], F32, name=f"rbp{b}{mc}", tag="aug")
                    nc.tensor.matmul(rbp[:], onesr_sb[:], rstd[:],
                                     start=True, stop=True)
                    rbc = spool.tile([P, 512], BF16, name=f"rbc{b}{mc}", tag="rbc")
                    nc.vector.tensor_copy(rbc[:], rbp[:])
                    st["r2"][mc] = r2
                    st["rbc"][mc] = rbc

                def k_chain(ec):
                    es = slice(ec * P, (ec + 1) * P)
                    kp = psum.tile([P, 512], F32, name=f"kp{b}{ec}", tag="ps")
                    for dc in range(DC):
                        nc.tensor.matmul(kp[:], wck_sb[:, dc, es],
                                         st["ctxt"][:, dc, :],
                                         start=(dc == 0), stop=(dc == DC - 1))
                    nc.vector.tensor_copy(st["k"][:, ec, :], kp[:])

                def v_chain(lc):
                    ls = slice(lc * P, (lc + 1) * P)
                    vp = psum.tile([P, 512], F32, name=f"vp{b}{lc}", tag="ps")
                    for dc in range(DC):
                        nc.tensor.matmul(vp[:], st["ctxt"][:, dc, ls],
                                         wcv_sb[:, dc, :],
                                         start=(dc == 0), stop=(dc == DC - 1))
                    nc.vector.tensor_copy(
                        st["v"][:, lc, :, 0:HD],
                        vp[:].rearrange("p (h d) -> p h d", d=HD))

                def q_chain(ec, mc):
                    es = slice(ec * P, (ec + 1) * P)
                    ms = slice(mc * 512, (mc + 1) * 512)
                    qp = psum.tile([P, 512], F32, name=f"qp{b}{ec}{mc}", tag="ps")
                    for cc in range(CC):
                        nc.tensor.matmul(qp[:], wq_sb[:, cc, es],
                                         st["xbf"][:, cc, ms],
                                         start=(cc == 0), stop=False)
                    nc.tensor.matmul(qp[:], qr2_sb[:, es], st["r2"][mc][:],
                                     start=False, stop=True)
                    nc.vector.tensor_tensor(st["q"][:, ec, mc, :], qp[:],
                                            st["rbc"][mc][:],
                                            op=mybir.AluOpType.mult)

                def sc_exp_group(mc, j):
                    if mc not in st["den8"]:
                        st["den8"][mc] = spool.tile([NH, 512], BF16,
                                                    name=f"den8{b}{mc}", tag="den8")
                        st["asb"][mc] = [None] * NH
                    ps_h = []
                    for lc in range(LC):
                        # two heads' scores in one 2-bank PSUM tile -> one exp
                        sc2 = psc.tile([P, 1024], F32,
                                       name=f"sc{b}{mc}{j}{lc}", tag="sc")
                        for hh in range(2):
                            po = hh * HD
                            nc.tensor.matmul(
                                sc2[:, hh * 512:(hh + 1) * 512],
                                st["k"][po:po + HD, j, lc * P:(lc + 1) * P],
                                st["q"][po:po + HD, j, mc, :],
                                start=True, stop=True)
                        pt = ppool.tile([P, 1024], BF16,
                                        name=f"pt{b}{mc}{j}{lc}", tag="pt")
                        nc.scalar.activation(
                            pt[:], sc2[:],
                            mybir.ActivationFunctionType.Exp,
                            bias=st["mb"][:, lc:lc + 1])
                        ps_h.append(pt)
                    return ps_h

                def attnv_group(mc, j, ps_h):
                    for hh in range(2):
                        h = 2 * j + hh
                        aug = paug.tile([HD + 1, 512], F32,
                                        name=f"aug{b}{mc}{j}{hh}", tag="aug")
                        for lc in range(LC):
                            nc.tensor.matmul(aug[:], st["v"][:, lc, h, :],
                                             ps_h[lc][:, hh * 512:(hh + 1) * 512],
                                             start=(lc == 0), stop=(lc == LC - 1))
                        asb = ppool.tile([HD + 1, 512], BF16,
                                         name=f"asb{b}{mc}{j}{hh}", tag="asb",
                                         bufs=10)
                        nc.vector.tensor_copy(asb[:], aug[:])
                        nc.sync.dma_start(st["den8"][mc][h:h + 1, :],
                                          asb[HD:HD + 1, :])
                        st["asb"][mc][h] = asb

                def norm(mc):
                    rcp8 = spool.tile([NH, 512], BF16, name=f"rcp8{b}{mc}", tag="rcp8")
                    with nc.allow_low_precision(reason="softmax denominators; error damped by residual"):
                        nc.vector.reciprocal(rcp8[:], st["den8"][mc][:])
                    rcp8_d = dpool.tile([NH, 512], BF16, name=f"rcpd{b}{mc}", tag="rcpd")
                    nc.sync.dma_start(rcp8_d[:], rcp8[:])
                    for h in range(NH):
                        j, hh = h // 2, h % 2
                        po = hh * HD
                        rcb = spool.tile([HD, 512], BF16,
                                         name=f"rcb{b}{mc}{h}", tag="rcb")
                        nc.sync.dma_start(
                            rcb[:], rcp8_d[h:h + 1, :].to_broadcast((HD, 512)))
                        nc.vector.tensor_tensor(
                            st["an"][po:po + HD, j, mc, :],
                            st["asb"][mc][h][0:HD, :], rcb[:],
                            op=mybir.AluOpType.mult)

                def outproj(cc, mc):
                    ms = slice(mc * 512, (mc + 1) * 512)
                    cs = slice(cc * P, (cc + 1) * P)
                    op = psum.tile([P, 512], F32, name=f"op{b}{cc}{mc}", tag="ps")
                    for ec in range(CC):
                        nc.tensor.matmul(op[:], wo_sb[:, ec, cs],
                                         st["an"][:, ec, mc, :],
                                         start=(ec == 0), stop=(ec == CC - 1))
                    y_sb = xpool.tile([P, 512], F32, name=f"y{b}{cc}{mc}",
                                      tag="y", bufs=4)
                    nc.vector.tensor_tensor(y_sb[:], op[:],
                                            st["x"][:, cc, ms],
                                            op=mybir.AluOpType.add)
                    nc.sync.dma_start(
                        yd.ap()[b][cc * P:(cc + 1) * P, mc * 512:(mc + 1) * 512],
                        y_sb[:])

                return dict(loads=loads, stats=stats, k_chain=k_chain,
                            v_chain=v_chain, q_chain=q_chain,
                            sc_exp_group=sc_exp_group, attnv_group=attnv_group,
                            norm=norm, outproj=outproj)

            # ---- software-pipelined emission (cross-batch modulo schedule) ----
            # PE fillers sit between ACT-bound score/exp groups and their
            # attn@v consumers; fillers are chosen to be independent of the
            # preceding normalize latency.
            E = [make_batch(b) for b in range(BPC)]
            E[0]["loads"]()
            E[0]["stats"](0)
            E[0]["stats"](1)
            for lc in range(LC):
                E[0]["v_chain"](lc)
            E[0]["k_chain"](0)
            E[0]["q_chain"](0, 0)
            E[0]["q_chain"](0, 1)
            for j in range(NH // 2):          # b0 attn mc0
                ps_h = E[0]["sc_exp_group"](0, j)
                if j < 3:
                    E[0]["k_chain"](j + 1)
                    E[0]["q_chain"](j + 1, 0)
                    E[0]["q_chain"](j + 1, 1)
                else:
                    E[1]["loads"]()
                E[0]["attnv_group"](0, j, ps_h)
            for j in range(NH // 2):          # b0 attn mc1; b1 early fillers
                ps_h = E[0]["sc_exp_group"](1, j)
                if j == 0:
                    E[1]["stats"](0)
                    E[1]["stats"](1)
                elif j == 1:
                    E[1]["v_chain"](0)
                    E[1]["v_chain"](1)
                elif j == 2:
                    E[1]["v_chain"](2)
                    E[1]["v_chain"](3)
                else:
                    E[1]["k_chain"](0)
                    E[1]["q_chain"](0, 0)
                    E[1]["q_chain"](0, 1)
                E[0]["attnv_group"](1, j, ps_h)
                if j == 0:
                    E[0]["norm"](0)   # delayed: keeps the reciprocal behind
                                      # this group's evictions in the DVE stream
            for j in range(NH // 2):          # b1 attn mc0; b0 outproj mc0 fillers
                ps_h = E[1]["sc_exp_group"](0, j)
                E[0]["outproj"](j, 0)
                if j < 3:
                    E[1]["k_chain"](j + 1)
                    E[1]["q_chain"](j + 1, 0)
                    E[1]["q_chain"](j + 1, 1)
                E[1]["attnv_group"](0, j, ps_h)
                if j == 0:
                    E[0]["norm"](1)
            for j in range(NH // 2):          # b1 attn mc1; outproj fillers
                ps_h = E[1]["sc_exp_group"](1, j)
                E[0]["outproj"](j, 1)
                if j > 0:
                    E[1]["outproj"](j - 1, 0)
                E[1]["attnv_group"](1, j, ps_h)
                if j == 0:
                    E[1]["norm"](0)
            E[1]["outproj"](3, 0)
            E[1]["norm"](1)
            for cc in range(CC):
                E[1]["outproj"](cc, 1)
    nc.compile()
    return nc


def _get_nc():
    global _NC_CACHE
    if _NC_CACHE is None:
        _NC_CACHE = _build()
    return _NC_CACHE


def kernel(x, context, context_mask, ln_w, ln_b, Wq, Wk, Wv, Wo, Wctx):
    x = np.asarray(x, np.float32)
    context = np.asarray(context, np.float32)
    context_mask = np.asarray(context_mask)
    ln_w = np.asarray(ln_w, np.float32)
    ln_b = np.asarray(ln_b, np.float32)
    Wq = np.asarray(Wq, np.float32)
    Wk = np.asarray(Wk, np.float32)
    Wv = np.asarray(Wv, np.float32)
    Wo = np.asarray(Wo, np.float32)
    Wctx = np.asarray(Wctx, np.float32)

    scale = HD ** -0.5
    wq_f = Wq * (ln_w[None, :] * scale)          # [E, C] ln scale + attn scale folded
    wq_t = np.ascontiguousarray(wq_f.T).astype(BF)
    q_r2 = np.stack([wq_f.sum(1), (Wq * scale) @ ln_b]).astype(BF)   # [2, E]
    wck_t = np.ascontiguousarray((Wk @ Wctx).T).astype(BF)           # [768, 512]
    wcv_t = np.ascontiguousarray((Wv @ Wctx).T).astype(BF)
    wo_t = np.ascontiguousarray(Wo.T).astype(BF)

    xr = x.reshape(NCORES, BPC, C, N)
    xbf = xr.astype(BF)
    ctxt = np.ascontiguousarray(
        context.transpose(0, 2, 1)).astype(BF).reshape(NCORES, BPC, CTX_DIM, L)
    mb = ((~context_mask).astype(np.float32) * MASK_NEG).reshape(NCORES, BPC, L)

    in_maps = [
        {"x": np.ascontiguousarray(xr[c]), "xbf": np.ascontiguousarray(xbf[c]),
         "ctxt": np.ascontiguousarray(ctxt[c]),
         "mb": np.ascontiguousarray(mb[c]), "wq_t": wq_t, "wck_t": wck_t,
         "wcv_t": wcv_t, "wo_t": wo_t, "q_r2": q_r2}
        for c in range(NCORES)
    ]
    res = run_bass_kernel_spmd(_get_nc(), in_maps, core_ids=list(range(NCORES)))
    y = np.stack([r["y"] for r in res.results])          # [8, 2, C, N]
    return y.reshape(B, C, H, W)
